# revision 16
# baseline (speedup 1.0000x reference)
"""Trainium2 Bass kernel for the YOLO-style grid loss (nn_Loss_12326556139840).

Pure data parallel over 8 NeuronCores; each core handles 2048 batch rows
(100,352 grid cells). The host marshals inputs into a planar, dim-major
channel layout so that every hot f16 DVE op is dense step-1 (2x/4x perf
mode). All SBUF tiles are flat 2-dim [128, k*N] (multi-dim APs defeat the
DVE perf-mode detector -> 1x fallback).

    DRAM per core: data[NG=2, 128, 59*N] f16 (N=392 cells/partition/group).
    Plane order (each plane = N cells):
      0-3   px0,px1,py0,py1      4-7   pw0,pw1,ph0,ph1
      8-11  tx0,tx1,ty0,ty1      12-15 tw0,tw1,th0,th1
      16-17 pc0,pc1              18    obj (= target conf, both boxes)
      19-38 p class              39-58 t class

Branchless math (validated vs f64 reference, rel err ~2.5e-5 measured):
  - IoU overlap at 2x scale: iw = relu(min(2pw,2tw,(pw+tw)-2|px-tx|))/2
  - responsibility via cross-compare i1*u0 > i0*u1 in f32 (f16 compare
    flips near-tie cells -> 1.8e-3 error; f32 keeps ~1e-5)
  - argmax/argmin tie semantics (resp=nonresp=0) preserved exactly
  - selects as {0,1}-mask blends: sel = a0 + m*(a1-a0)
  - fxy = frac(7*xy) with integer -> 1, via 2^23 magic (DVE internal fp32)
  - loss weights folded into ACT Square scale = sqrt(w)
Engine split: DVE elementwise, ACT abs/sqrt + square-accumulate, GPSIMD
the 20-plane class subtract.
"""

import os

import numpy as np

import concourse.bacc as bacc
import concourse.tile as tile
from concourse import mybir
from concourse.bass_utils import run_bass_kernel_spmd

F32 = mybir.dt.float32
F16 = mybir.dt.float16
Alu = mybir.AluOpType
Act = mybir.ActivationFunctionType

B = 16384
NCORES = 8
BPC = B // NCORES           # 2048 rows per core
P = 128
CELLS = BPC * 49            # 100352 cells per core
NTOT = CELLS // P           # 784 cells per partition
NG = 2                      # groups per core
N = NTOT // NG              # 392 cells per partition per group
NPL = 78                    # planes per cell

EPS = 1e-7
MAGIC = 8388608.0           # 2^23: round-to-nearest in the DVE's internal fp32
SQRT5 = float(np.sqrt(5.0))
SQRTH = float(np.sqrt(0.5))

# plane offsets
PXY, PWH, TXY, TWH, PC, OBJ, PCL, TCL = 0, 4, 8, 12, 16, 18, 38, 58

# host-side channel permutation: planes -> source channel (0-29 pre, 30-59 tgt)
PERM = (
    [0, 5, 1, 6,  2, 7, 3, 8,  30, 35, 31, 36,  32, 37, 33, 38,  4, 9]
    + [34] * 20
    + list(range(10, 30))
    + list(range(40, 60))
)

_DEBUG_ACC = bool(os.environ.get("BASSK_DEBUG"))


def _build():
    nc = bacc.Bacc()
    data_d = nc.declare_dram_parameter("data", [NG, P, NPL * N], F16, isOutput=False)
    out_shape = [P, NG * 5]
    out_d = nc.declare_dram_parameter("out", out_shape, F32, isOutput=True)

    with tile.TileContext(nc) as tc:
        with (
            tc.tile_pool(name="din", bufs=2) as din,
            tc.tile_pool(name="w4", bufs=8) as w4p,
            tc.tile_pool(name="w2f", bufs=6) as w2fp,
            tc.tile_pool(name="w2", bufs=10) as w2p,
            tc.tile_pool(name="w1", bufs=10) as w1p,
            tc.tile_pool(name="dm", bufs=2) as dmp,
            tc.tile_pool(name="acc", bufs=1) as accp,
        ):
            v = nc.vector
            s = nc.scalar
            g = nc.gpsimd

            NCOL = NG * 5
            acc32 = accp.tile([P, NCOL], F32, tag="acc")
            v.memset(acc32, 0.0)
            eps_b = accp.tile([P, 1], F32, tag="epsb")
            v.memset(eps_b, EPS)
            eps5_b = accp.tile([P, 1], F32, tag="eps5b")
            v.memset(eps5_b, 5.0 * EPS)

            for gi in range(NG):
                col = gi * 5
                dt = din.tile([P, TCL * N], F16, tag="d", name="d")
                # box/obj planes land first so the IoU chain starts early
                nc.sync.dma_start(out=dt[:, 0 : 19 * N],
                                  in_=data_d[gi, :, 0 : 19 * N])
                nc.sync.dma_start(out=dt[:, 19 * N : TCL * N],
                                  in_=data_d[gi, :, 19 * N : TCL * N])
                # class diff p-t computed by the DMA engine: host ships -tcl,
                # SWDGE CCE-add accumulates it onto the pcl region. CCE caps
                # descriptors at 2048 elements -> 5-plane (1960) chunks.
                for c in range(0, 20, 5):
                    g.dma_start(out=dt[:, (PCL + c) * N : (PCL + c + 5) * N],
                                in_=data_d[gi, :, (TCL + c) * N : (TCL + c + 5) * N],
                                accum_op=Alu.add)

                def fl(i, k=1):
                    return dt[:, i * N : (i + k) * N]

                def w4t():
                    return w4p.tile([P, 4 * N], F16, tag="w4", name="w4")

                def w2t():
                    return w2p.tile([P, 2 * N], F16, tag="w2", name="w2")

                def w2f():
                    return w2fp.tile([P, 2, N], F32, tag="w2f", name="w2f")

                def w1t():
                    return w1p.tile([P, N], F16, tag="w1", name="w1")

                obj = fl(OBJ)

                # ---------------- IoU (f16 chain, dim-major) ----------------
                dxy4 = w4t()
                v.tensor_sub(dxy4, fl(PXY, 4), fl(TXY, 4))
                ad24 = w4t()
                s.activation(ad24, dxy4, Act.Abs, scale=2.0)
                swh4 = w4t()
                v.tensor_add(swh4, fl(PWH, 4), fl(TWH, 4))
                e4 = w4t()
                v.tensor_sub(e4, swh4, ad24)
                mwh4 = w4t()
                v.tensor_tensor(mwh4, fl(PWH, 4), fl(TWH, 4), op=Alu.min)
                m24 = w4t()
                v.scalar_tensor_tensor(m24, mwh4, 2.0, e4, op0=Alu.mult, op1=Alu.min)
                iwh4 = w4t()
                v.tensor_scalar(iwh4, m24, 0.0, 0.5, op0=Alu.max, op1=Alu.mult)
                # iwh4 planes: [iw0, iw1, ih0, ih1]

                # ------------- responsibility (f32 products, 1x anyway) -------------
                inter2 = w2f()
                v.tensor_mul(inter2[:].rearrange("p c n -> p (c n)"),
                             iwh4[:, 0 : 2 * N], iwh4[:, 2 * N : 4 * N])
                ap2 = w2f()
                v.tensor_mul(ap2[:].rearrange("p c n -> p (c n)"),
                             fl(PWH + 2, 2), fl(PWH, 2))
                at2 = w2f()
                v.tensor_mul(at2[:].rearrange("p c n -> p (c n)"),
                             fl(TWH + 2, 2), fl(TWH, 2))
                sa2 = w2f()
                v.tensor_add(sa2, ap2, at2)
                un2 = w2f()
                v.tensor_sub(un2, sa2, inter2)
                ie2 = w2f()
                v.tensor_single_scalar(ie2, inter2, EPS, op=Alu.add)
                cr2 = w2fp.tile([P, 2, N], F32, tag="w2f", name="cr2")
                v.tensor_mul(cr2, ie2[:], un2[:, ::-1, :])
                # one op: [respc, nonrc] = [c1>c0, c0>c1]
                rp2 = w2t()
                v.tensor_tensor(rp2[:].rearrange("p (c n) -> p c n", c=2),
                                cr2[:, ::-1, :], cr2[:], op=Alu.is_gt)
                respc = rp2[:, 0:N]
                nonrc = rp2[:, N : 2 * N]

                # ---------------- mask planes ----------------
                w2m = w2t()          # [b0w, b1w] = obj * [1-respc, respc]
                b0w = w2m[:, 0:N]
                b1w = w2m[:, N : 2 * N]
                v.tensor_mul(b1w, respc, obj)
                v.tensor_sub(b0w, obj, b1w)
                noobj = w1t()
                v.tensor_scalar(noobj, obj, -1.0, 1.0, op0=Alu.mult, op1=Alu.add)
                ncp = w2t()          # conf targets [nc0, nc1]
                v.tensor_mul(ncp[:, 0:N], nonrc, obj)
                v.tensor_sub(ncp[:, N : 2 * N], obj, ncp[:, 0:N])
                h2 = w2t()           # half-weight masks, pre-scaled sqrt(.5)
                v.tensor_add(h2[:, 0:N], b1w, noobj)
                v.tensor_add(h2[:, N : 2 * N], b0w, noobj)
                hs2 = w2t()
                v.tensor_single_scalar(hs2, h2, SQRTH, op=Alu.mult)
                wm5 = w2t()          # xy mask pre-scaled sqrt(5)
                v.tensor_single_scalar(wm5, w2m, SQRT5, op=Alu.mult)

                # masked diffs of wh/xy/conf collect into one tile -> one Square
                dmall = dmp.tile([P, 12 * N], F16, tag="dmall", name="dmall")

                # ---------------- conf ----------------
                dc2 = w2t()
                v.tensor_sub(dc2, fl(PC, 2), ncp)
                v.tensor_mul(dmall[:, 8 * N : 10 * N], dc2, w2m)
                v.tensor_mul(dmall[:, 10 * N : 12 * N], dc2, hs2)

                # ---------------- wh (sqrt(5) folded into ACT scale) ----------------
                sp4 = w4t()
                s.activation(sp4, fl(PWH, 4), Act.Sqrt, scale=5.0, bias=eps5_b)
                st4 = w4t()
                s.activation(st4, fl(TWH, 4), Act.Sqrt, scale=5.0, bias=eps5_b)
                dwh4 = w4t()
                v.tensor_sub(dwh4, sp4, st4)
                v.tensor_mul(dmall[:, 0 : 2 * N], dwh4[:, 0 : 2 * N], w2m)
                v.tensor_mul(dmall[:, 2 * N : 4 * N], dwh4[:, 2 * N : 4 * N], w2m)


                # ------- class (obj pre-replicated x20; halves pipeline DVE/ACT) -------
                for hi in range(2):
                    dcl = fl(PCL + 10 * hi, 10)
                    v.tensor_mul(dcl, dcl, fl(OBJ + 10 * hi, 10))
                    s.activation(dcl, dcl, Act.Square,
                                 accum_out=acc32[:, col + 1 + hi : col + 2 + hi])

                # ---------------- xy ----------------
                dtp = w2t()          # [tx1-tx0, ty1-ty0]
                v.tensor_sub(dtp[:, 0:N], fl(TXY + 1), fl(TXY))
                v.tensor_sub(dtp[:, N : 2 * N], fl(TXY + 3), fl(TXY + 2))
                nr2 = w2t()
                v.tensor_copy(nr2[:, 0:N], nonrc)
                v.tensor_copy(nr2[:, N : 2 * N], nonrc)
                nd2 = w2t()
                v.tensor_mul(nd2, dtp, nr2)
                xynr = w2t()         # [x_nr, y_nr]
                v.tensor_add(xynr[:, 0:N], fl(TXY), nd2[:, 0:N])
                v.tensor_add(xynr[:, N : 2 * N], fl(TXY + 2), nd2[:, N : 2 * N])
                u2 = w2t()
                v.tensor_single_scalar(u2, xynr, 7.0, op=Alu.mult)
                r2 = w2t()
                v.tensor_scalar(r2, u2, MAGIC, MAGIC, op0=Alu.add, op1=Alu.subtract)
                fr2 = w2t()
                v.tensor_sub(fr2, u2, r2)
                le2 = w2t()
                v.tensor_single_scalar(le2, fr2, 0.0, op=Alu.is_le)
                fxy2 = w2t()
                v.tensor_add(fxy2, fr2, le2)
                fx4 = w4t()          # [fx, fx, fy, fy]
                s.copy(fx4[:, 0:N], fxy2[:, 0:N])
                s.copy(fx4[:, N : 2 * N], fxy2[:, 0:N])
                s.copy(fx4[:, 2 * N : 3 * N], fxy2[:, N : 2 * N])
                s.copy(fx4[:, 3 * N : 4 * N], fxy2[:, N : 2 * N])
                d4xy = w4t()
                v.tensor_sub(d4xy, fl(PXY, 4), fx4)
                v.tensor_mul(dmall[:, 4 * N : 6 * N], d4xy[:, 0 : 2 * N], wm5)
                v.tensor_mul(dmall[:, 6 * N : 8 * N], d4xy[:, 2 * N : 4 * N], wm5)
                s.activation(dmall, dmall, Act.Square,
                             accum_out=acc32[:, col : col + 1])

            # ---------------- store raw accumulator columns; host sums ----------------
            nc.sync.dma_start(out=out_d[:], in_=acc32[:])

    nc.compile()
    return nc


_NC_CACHE = None
_LAST_ACC = None


def _marshal(pre: np.ndarray, target: np.ndarray) -> np.ndarray:
    """Build [NCORES, NG, P, NPL*N] f16 planar layout."""
    p = np.ascontiguousarray(pre, dtype=np.float32).reshape(B, 49, 30)
    t = np.ascontiguousarray(target, dtype=np.float32).reshape(B, 49, 30)
    s = np.concatenate([p, t], axis=2).astype(np.float16)      # [B, 49, 60]
    # rows per partition = 16; cells per group per partition = N (8 rows)
    s = s.reshape(NCORES, P, NG, N, 60)
    s = s.transpose(0, 2, 1, 4, 3)                             # [8, NG, P, 60, N]
    s = np.ascontiguousarray(s[:, :, :, PERM, :])              # [8, NG, P, NPL, N]
    np.negative(s[:, :, :, TCL:NPL, :], out=s[:, :, :, TCL:NPL, :])
    return s.reshape(NCORES, NG, P, NPL * N)


def kernel(pre: np.ndarray, target: np.ndarray) -> np.ndarray:
    global _NC_CACHE, _LAST_ACC
    if _NC_CACHE is None:
        _NC_CACHE = _build()
    nc = _NC_CACHE

    data = _marshal(pre, target)
    in_maps = [{"data": data[i]} for i in range(NCORES)]
    res = run_bass_kernel_spmd(nc, in_maps, core_ids=list(range(NCORES)))
    if _DEBUG_ACC:
        _LAST_ACC = [r["out"] for r in res.results]
    total = 0.0
    for r in res.results:
        total += float(np.sum(r["out"].astype(np.float64)))
    return np.float32(total / B)


# revision 17
# speedup vs baseline: 1.0320x; 1.0320x over previous
"""Trainium2 Bass kernel for the YOLO-style grid loss (nn_Loss_12326556139840).

Pure data parallel over 8 NeuronCores; each core handles 2048 batch rows
(100,352 grid cells). The host marshals inputs into a planar, dim-major
channel layout so that every hot f16 DVE op is dense step-1 (2x/4x perf
mode). All SBUF tiles are flat 2-dim [128, k*N] (multi-dim APs defeat the
DVE perf-mode detector -> 1x fallback).

    DRAM per core: data[NG=2, 128, 59*N] f16 (N=392 cells/partition/group).
    Plane order (each plane = N cells):
      0-3   px0,px1,py0,py1      4-7   pw0,pw1,ph0,ph1
      8-11  tx0,tx1,ty0,ty1      12-15 tw0,tw1,th0,th1
      16-17 pc0,pc1              18    obj (= target conf, both boxes)
      19-38 p class              39-58 t class

Branchless math (validated vs f64 reference, rel err ~2.5e-5 measured):
  - IoU overlap at 2x scale: iw = relu(min(2pw,2tw,(pw+tw)-2|px-tx|))/2
  - responsibility via cross-compare i1*u0 > i0*u1 in f32 (f16 compare
    flips near-tie cells -> 1.8e-3 error; f32 keeps ~1e-5)
  - argmax/argmin tie semantics (resp=nonresp=0) preserved exactly
  - selects as {0,1}-mask blends: sel = a0 + m*(a1-a0)
  - fxy = frac(7*xy) with integer -> 1, via 2^23 magic (DVE internal fp32)
  - loss weights folded into ACT Square scale = sqrt(w)
Engine split: DVE elementwise, ACT abs/sqrt + square-accumulate, GPSIMD
the 20-plane class subtract.
"""

import os

import numpy as np

import concourse.bacc as bacc
import concourse.tile as tile
from concourse import mybir
from concourse.bass_utils import run_bass_kernel_spmd

F32 = mybir.dt.float32
F16 = mybir.dt.float16
Alu = mybir.AluOpType
Act = mybir.ActivationFunctionType

B = 16384
NCORES = 8
BPC = B // NCORES           # 2048 rows per core
P = 128
CELLS = BPC * 49            # 100352 cells per core
NTOT = CELLS // P           # 784 cells per partition
NG = 2                      # groups per core
N = NTOT // NG              # 392 cells per partition per group
NPL = 78                    # planes per cell

EPS = 1e-7
MAGIC = 8388608.0           # 2^23: round-to-nearest in the DVE's internal fp32
SQRT5 = float(np.sqrt(5.0))
SQRTH = float(np.sqrt(0.5))

# plane offsets
PXY, PWH, TXY, TWH, PC, OBJ, PCL, TCL = 0, 4, 8, 12, 16, 18, 38, 58

# host-side channel permutation: planes -> source channel (0-29 pre, 30-59 tgt)
PERM = (
    [0, 5, 1, 6,  2, 7, 3, 8,  30, 35, 31, 36,  32, 37, 33, 38,  4, 9]
    + [34] * 20
    + list(range(10, 30))
    + list(range(40, 60))
)

_DEBUG_ACC = bool(os.environ.get("BASSK_DEBUG"))


def _build():
    nc = bacc.Bacc()
    data_d = nc.declare_dram_parameter("data", [NG, P, NPL * N], F16, isOutput=False)
    out_shape = [P, NG * 5]
    out_d = nc.declare_dram_parameter("out", out_shape, F32, isOutput=True)

    with tile.TileContext(nc) as tc:
        with (
            tc.tile_pool(name="din", bufs=2) as din,
            tc.tile_pool(name="w4", bufs=8) as w4p,
            tc.tile_pool(name="w2f", bufs=6) as w2fp,
            tc.tile_pool(name="w2", bufs=10) as w2p,
            tc.tile_pool(name="w1", bufs=10) as w1p,
            tc.tile_pool(name="dm", bufs=2) as dmp,
            tc.tile_pool(name="acc", bufs=1) as accp,
        ):
            v = nc.vector
            s = nc.scalar
            g = nc.gpsimd

            NCOL = NG * 5
            acc32 = accp.tile([P, NCOL], F32, tag="acc")
            v.memset(acc32, 0.0)
            eps_b = accp.tile([P, 1], F32, tag="epsb")
            v.memset(eps_b, EPS)
            eps5_b = accp.tile([P, 1], F32, tag="eps5b")
            v.memset(eps5_b, 5.0 * EPS)

            for gi in range(NG):
                col = gi * 5
                dt = din.tile([P, NPL * N], F16, tag="d", name="d")
                # box/obj planes land first so the IoU chain starts early
                nc.sync.dma_start(out=dt[:, 0 : 19 * N],
                                  in_=data_d[gi, :, 0 : 19 * N])
                nc.sync.dma_start(out=dt[:, 19 * N : NPL * N],
                                  in_=data_d[gi, :, 19 * N : NPL * N])

                def fl(i, k=1):
                    return dt[:, i * N : (i + k) * N]

                def w4t():
                    return w4p.tile([P, 4 * N], F16, tag="w4", name="w4")

                def w2t():
                    return w2p.tile([P, 2 * N], F16, tag="w2", name="w2")

                def w2f():
                    return w2fp.tile([P, 2, N], F32, tag="w2f", name="w2f")

                def w1t():
                    return w1p.tile([P, N], F16, tag="w1", name="w1")

                obj = fl(OBJ)

                # ---------------- IoU (f16 chain, dim-major) ----------------
                dxy4 = w4t()
                v.tensor_sub(dxy4, fl(PXY, 4), fl(TXY, 4))
                ad24 = w4t()
                s.activation(ad24, dxy4, Act.Abs, scale=2.0)
                swh4 = w4t()
                v.tensor_add(swh4, fl(PWH, 4), fl(TWH, 4))
                e4 = w4t()
                v.tensor_sub(e4, swh4, ad24)
                mwh4 = w4t()
                v.tensor_tensor(mwh4, fl(PWH, 4), fl(TWH, 4), op=Alu.min)
                m24 = w4t()
                v.scalar_tensor_tensor(m24, mwh4, 2.0, e4, op0=Alu.mult, op1=Alu.min)
                iwh4 = w4t()
                v.tensor_scalar(iwh4, m24, 0.0, 0.5, op0=Alu.max, op1=Alu.mult)
                # iwh4 planes: [iw0, iw1, ih0, ih1]

                # ------------- responsibility (f32 products, 1x anyway) -------------
                inter2 = w2f()
                v.tensor_mul(inter2[:].rearrange("p c n -> p (c n)"),
                             iwh4[:, 0 : 2 * N], iwh4[:, 2 * N : 4 * N])
                ap2 = w2f()
                v.tensor_mul(ap2[:].rearrange("p c n -> p (c n)"),
                             fl(PWH + 2, 2), fl(PWH, 2))
                at2 = w2f()
                v.tensor_mul(at2[:].rearrange("p c n -> p (c n)"),
                             fl(TWH + 2, 2), fl(TWH, 2))
                sa2 = w2f()
                v.tensor_add(sa2, ap2, at2)
                un2 = w2f()
                v.tensor_sub(un2, sa2, inter2)
                ie2 = w2f()
                v.tensor_single_scalar(ie2, inter2, EPS, op=Alu.add)
                cr2 = w2fp.tile([P, 2, N], F32, tag="w2f", name="cr2")
                v.tensor_mul(cr2, ie2[:], un2[:, ::-1, :])
                # one op: [respc, nonrc] = [c1>c0, c0>c1]
                rp2 = w2t()
                v.tensor_tensor(rp2[:].rearrange("p (c n) -> p c n", c=2),
                                cr2[:, ::-1, :], cr2[:], op=Alu.is_gt)
                respc = rp2[:, 0:N]
                nonrc = rp2[:, N : 2 * N]

                # ---------------- mask planes ----------------
                w2m = w2t()          # [b0w, b1w] = obj * [1-respc, respc]
                b0w = w2m[:, 0:N]
                b1w = w2m[:, N : 2 * N]
                v.tensor_mul(b1w, respc, obj)
                v.tensor_sub(b0w, obj, b1w)
                noobj = w1t()
                v.tensor_scalar(noobj, obj, -1.0, 1.0, op0=Alu.mult, op1=Alu.add)
                ncp = w2t()          # conf targets [nc0, nc1]
                v.tensor_mul(ncp[:, 0:N], nonrc, obj)
                v.tensor_sub(ncp[:, N : 2 * N], obj, ncp[:, 0:N])
                h2 = w2t()           # half-weight masks, pre-scaled sqrt(.5)
                v.tensor_add(h2[:, 0:N], b1w, noobj)
                v.tensor_add(h2[:, N : 2 * N], b0w, noobj)
                hs2 = w2t()
                v.tensor_single_scalar(hs2, h2, SQRTH, op=Alu.mult)
                wm5 = w2t()          # xy mask pre-scaled sqrt(5)
                v.tensor_single_scalar(wm5, w2m, SQRT5, op=Alu.mult)

                # masked diffs of wh/xy/conf collect into one tile -> one Square
                dmall = dmp.tile([P, 12 * N], F16, tag="dmall", name="dmall")

                # ---------------- conf ----------------
                dc2 = w2t()
                v.tensor_sub(dc2, fl(PC, 2), ncp)
                v.tensor_mul(dmall[:, 8 * N : 10 * N], dc2, w2m)
                v.tensor_mul(dmall[:, 10 * N : 12 * N], dc2, hs2)

                # ---------------- wh (sqrt(5) folded into ACT scale) ----------------
                sp4 = w4t()
                s.activation(sp4, fl(PWH, 4), Act.Sqrt, scale=5.0, bias=eps5_b)
                st4 = w4t()
                s.activation(st4, fl(TWH, 4), Act.Sqrt, scale=5.0, bias=eps5_b)
                dwh4 = w4t()
                v.tensor_sub(dwh4, sp4, st4)
                v.tensor_mul(dmall[:, 0 : 2 * N], dwh4[:, 0 : 2 * N], w2m)
                v.tensor_mul(dmall[:, 2 * N : 4 * N], dwh4[:, 2 * N : 4 * N], w2m)


                # ------- class (obj pre-replicated x20; halves pipeline DVE/ACT) -------
                for hi in range(2):
                    dcl = fl(PCL + 10 * hi, 10)
                    # host ships -tcl, so the class diff is an add
                    v.tensor_add(dcl, dcl, fl(TCL + 10 * hi, 10))
                    v.tensor_mul(dcl, dcl, fl(OBJ + 10 * hi, 10))
                    s.activation(dcl, dcl, Act.Square,
                                 accum_out=acc32[:, col + 1 + hi : col + 2 + hi])

                # ---------------- xy ----------------
                dtp = w2t()          # [tx1-tx0, ty1-ty0]
                v.tensor_sub(dtp[:, 0:N], fl(TXY + 1), fl(TXY))
                v.tensor_sub(dtp[:, N : 2 * N], fl(TXY + 3), fl(TXY + 2))
                nr2 = w2t()
                v.tensor_copy(nr2[:, 0:N], nonrc)
                v.tensor_copy(nr2[:, N : 2 * N], nonrc)
                nd2 = w2t()
                v.tensor_mul(nd2, dtp, nr2)
                xynr = w2t()         # [x_nr, y_nr]
                v.tensor_add(xynr[:, 0:N], fl(TXY), nd2[:, 0:N])
                v.tensor_add(xynr[:, N : 2 * N], fl(TXY + 2), nd2[:, N : 2 * N])
                u2 = w2t()
                v.tensor_single_scalar(u2, xynr, 7.0, op=Alu.mult)
                r2 = w2t()
                v.tensor_scalar(r2, u2, MAGIC, MAGIC, op0=Alu.add, op1=Alu.subtract)
                fr2 = w2t()
                v.tensor_sub(fr2, u2, r2)
                le2 = w2t()
                v.tensor_single_scalar(le2, fr2, 0.0, op=Alu.is_le)
                fxy2 = w2t()
                v.tensor_add(fxy2, fr2, le2)
                fx4 = w4t()          # [fx, fx, fy, fy]
                s.copy(fx4[:, 0:N], fxy2[:, 0:N])
                s.copy(fx4[:, N : 2 * N], fxy2[:, 0:N])
                s.copy(fx4[:, 2 * N : 3 * N], fxy2[:, N : 2 * N])
                s.copy(fx4[:, 3 * N : 4 * N], fxy2[:, N : 2 * N])
                d4xy = w4t()
                v.tensor_sub(d4xy, fl(PXY, 4), fx4)
                v.tensor_mul(dmall[:, 4 * N : 6 * N], d4xy[:, 0 : 2 * N], wm5)
                v.tensor_mul(dmall[:, 6 * N : 8 * N], d4xy[:, 2 * N : 4 * N], wm5)
                s.activation(dmall, dmall, Act.Square,
                             accum_out=acc32[:, col : col + 1])

            # ---------------- store raw accumulator columns; host sums ----------------
            nc.sync.dma_start(out=out_d[:], in_=acc32[:])

    nc.compile()
    return nc


_NC_CACHE = None
_LAST_ACC = None


def _marshal(pre: np.ndarray, target: np.ndarray) -> np.ndarray:
    """Build [NCORES, NG, P, NPL*N] f16 planar layout."""
    p = np.ascontiguousarray(pre, dtype=np.float32).reshape(B, 49, 30)
    t = np.ascontiguousarray(target, dtype=np.float32).reshape(B, 49, 30)
    s = np.concatenate([p, t], axis=2).astype(np.float16)      # [B, 49, 60]
    # rows per partition = 16; cells per group per partition = N (8 rows)
    s = s.reshape(NCORES, P, NG, N, 60)
    s = s.transpose(0, 2, 1, 4, 3)                             # [8, NG, P, 60, N]
    s = np.ascontiguousarray(s[:, :, :, PERM, :])              # [8, NG, P, NPL, N]
    np.negative(s[:, :, :, TCL:NPL, :], out=s[:, :, :, TCL:NPL, :])
    return s.reshape(NCORES, NG, P, NPL * N)


def kernel(pre: np.ndarray, target: np.ndarray) -> np.ndarray:
    global _NC_CACHE, _LAST_ACC
    if _NC_CACHE is None:
        _NC_CACHE = _build()
    nc = _NC_CACHE

    data = _marshal(pre, target)
    in_maps = [{"data": data[i]} for i in range(NCORES)]
    res = run_bass_kernel_spmd(nc, in_maps, core_ids=list(range(NCORES)))
    if _DEBUG_ACC:
        _LAST_ACC = [r["out"] for r in res.results]
    total = 0.0
    for r in res.results:
        total += float(np.sum(r["out"].astype(np.float64)))
    return np.float32(total / B)
